# revision 5
# baseline (speedup 1.0000x reference)
"""AntiSymmetricDGN on 8 TRN2 NeuronCores (Bass/Tile, SPMD) — v2.

Node-sharded graph parallel, per-core state TRANSPOSED in SBUF
[feat, pos] where pos is a PACKED node order (degree-aware window
packing, 196 windows x 32 cols = 6272 positions per core, 22 dead).

Per conv iteration:
  - table rows (node-major bf16 hw rows) staged per superblock into
    local ag bufs; two AllGathers -> shared tables A (positions <4096,
    8x4096 rows) and B (positions >=4096, 8x2176 rows).
  - SWDGE dma_gather by edge-slot (idx int16) -> g tiles [128,C,128].
  - segment-sum via PE chunk matmuls: lhsT = g[:, j, :] (128 edges x
    feat), rhs = scoef[:, j, :] (128 edges x 32 dst cols), accumulated
    per 512-col superblock in PSUM.
  - self-loop term dense: psy += gcn_w @ (selfco * h).
  - aW term, tanh, Euler update; next iteration's table staged in the
    same sb pass (gcn matmul + PE transpose + DMA).

Degree-aware packing targets (lo<=256, hi<=128|256) per window so
per-window chunk capacities are (2, 1|2) -> ~1-4% slot padding vs 23%
in v1. Self-loops excluded from the edge stream.
"""
import math
import os
import numpy as np

import concourse.bass as bass
from concourse import mybir, bacc
from concourse.bass_utils import run_bass_kernel_spmd
from concourse.tile import TileContext

# problem constants
N, E, IN, H, H2, OUT = 50000, 600000, 256, 128, 64, 40
EPS, GAMMA = 0.1, 0.1
NCORES = 8
NSH = N // NCORES          # 6250 real nodes per core
WCOL = 32
NW = 196                   # windows per core
PNSH = NW * WCOL           # 6272 packed positions
SBW = 16                   # windows per superblock
NSB = (NW + SBW - 1) // SBW    # 13 superblocks (last has 4 windows)
ASIDE = 128 * WCOL         # 4096 positions on side A
BSIDE = PNSH - ASIDE       # 2176 on side B
NAW = 128                  # A-side windows
CHUNK = 128

F32 = mybir.dt.float32
BF16 = mybir.dt.bfloat16
I16 = mybir.dt.int16
AF = mybir.ActivationFunctionType
ALU = mybir.AluOpType

GCAP = int(os.environ.get("K_GCAP", "8"))
SCRATCH = int(os.environ.get("K_SCRATCH", "49152"))


def _pack_side(deg_lo, deg_hi, nodes, nwin, cap_hi_list):
    """Pack `nodes` (indices) into nwin windows of <=32 nodes with
    per-window edge budgets (256 lo, cap_hi*128 hi). LPT greedy with
    normalized-min-room score + repair passes. Returns dict node->win."""
    nodes = np.asarray(nodes)
    dl_a = deg_lo[nodes].astype(np.float64)
    dh_a = deg_hi[nodes].astype(np.float64)
    order = np.argsort(-(dl_a + dh_a), kind="stable")
    tlo = np.full(nwin, 256.0)
    thi = np.array([128.0 * c for c in cap_hi_list], np.float64)
    rem_lo = tlo.copy()
    rem_hi = thi.copy()
    slots = np.full(nwin, 32, dtype=np.int64)
    assign = np.full(len(nodes), -1, np.int64)
    for i in order:
        dl, dh = dl_a[i], dh_a[i]
        rl = (rem_lo - dl) / tlo
        rh = (rem_hi - dh) / thi
        score = np.minimum(rl, rh)
        feas = (slots > 0) & (rem_lo >= dl) & (rem_hi >= dh)
        sc = np.where(feas, score, -np.inf)
        if not np.isfinite(sc).any():
            sc = np.where(slots > 0, score, -np.inf)
        w = int(np.argmax(sc))
        assign[i] = w
        rem_lo[w] -= dl
        rem_hi[w] -= dh
        slots[w] -= 1
    # repair: move offenders out of overloaded windows
    for _ in range(12):
        over = np.nonzero((rem_lo < 0) | (rem_hi < 0))[0]
        if len(over) == 0:
            break
        moved = False
        for w in over:
            members = np.nonzero(assign == w)[0]
            by_sz = members[np.argsort(-(dl_a[members] + dh_a[members]))]
            for i in by_sz:
                if rem_lo[w] >= 0 and rem_hi[w] >= 0:
                    break
                dl, dh = dl_a[i], dh_a[i]
                feas = (slots > 0) & (rem_lo >= dl) & (rem_hi >= dh)
                feas[w] = False
                if not feas.any():
                    continue
                cand = np.where(feas, np.minimum((rem_lo - dl) / tlo,
                                                 (rem_hi - dh) / thi), -np.inf)
                w2 = int(np.argmax(cand))
                assign[i] = w2
                rem_lo[w] += dl
                rem_hi[w] += dh
                slots[w] += 1
                rem_lo[w2] -= dl
                rem_hi[w2] -= dh
                slots[w2] -= 1
                moved = True
        if not moved:
            break
    return {int(nodes[i]): int(assign[i]) for i in range(len(nodes))}


def _prep_graph(edge_index):
    src = np.asarray(edge_index[0], dtype=np.int64)
    dst = np.asarray(edge_index[1], dtype=np.int64)
    loops = np.arange(N, dtype=np.int64)
    degL = np.bincount(np.concatenate([dst, loops]), minlength=N).astype(np.float32)
    dinv = (1.0 / np.sqrt(np.maximum(degL, 1e-12))).astype(np.float32)
    dinv[degL <= 0] = 0.0
    norm = (dinv[src] * dinv[dst]).astype(np.float32)
    selfco = (dinv * dinv).astype(np.float32)

    score = dst // NSH  # dst core of each edge
    nloc = dst % NSH
    sc_core = src // NSH

    # ---- round 1: pack by total degree to fix sides
    deg_tot = np.zeros((NCORES, NSH), np.int64)
    np.add.at(deg_tot, (score, nloc), 1)
    sideA = np.zeros((NCORES, NSH), bool)
    for c in range(NCORES):
        order = np.argsort(-deg_tot[c], kind="stable")
        rem = np.full(NW, 384.0)
        slots = np.full(NW, 32, np.int64)
        w_of = np.full(NSH, -1, np.int64)
        for n in order:
            d = deg_tot[c, n]
            s2 = np.where(slots > 0, rem - d, -np.inf)
            w = int(np.argmax(s2))
            w_of[n] = w
            rem[w] -= d
            slots[w] -= 1
        sideA[c] = w_of < NAW

    # edge lo-ness: src node's side on its core
    src_side_a = sideA[sc_core, src % NSH]

    # per-node (dst) lo/hi in-degree
    deg_lo = np.zeros((NCORES, NSH), np.int64)
    deg_hi = np.zeros((NCORES, NSH), np.int64)
    np.add.at(deg_lo, (score[src_side_a], nloc[src_side_a]), 1)
    np.add.at(deg_hi, (score[~src_side_a], nloc[~src_side_a]), 1)

    # hi-special windows: extra hi capacity. Estimate demand per side.
    spec_a = int(os.environ.get("K_SPECA", "8"))
    spec_b = int(os.environ.get("K_SPECB", "5"))
    caphi_a = [2 if w >= NAW - spec_a else 1 for w in range(NAW)]
    caphi_b = [2 if w >= (NW - NAW) - spec_b else 1 for w in range(NW - NAW)]

    # ---- round 2: pack each side with known lo/hi degrees
    win_of = np.full((NCORES, NSH), -1, np.int64)
    col_of = np.full((NCORES, NSH), -1, np.int64)
    for c in range(NCORES):
        na = _pack_side(deg_lo[c], deg_hi[c], np.nonzero(sideA[c])[0], NAW, caphi_a)
        nb = _pack_side(deg_lo[c], deg_hi[c], np.nonzero(~sideA[c])[0],
                        NW - NAW, caphi_b)
        raw = np.full(NSH, -1, np.int64)
        for n, w in na.items():
            raw[n] = w
        for n, w in nb.items():
            raw[n] = NAW + w
        # relabel windows within each side so per-core capacity profiles
        # align across cores (sort by needed chunks desc)
        wlo = np.zeros(NW)
        whi = np.zeros(NW)
        np.add.at(wlo, raw, deg_lo[c])
        np.add.at(whi, raw, deg_hi[c])
        key = (np.ceil(wlo / CHUNK) * 100 + np.ceil(whi / CHUNK)) * 1000 \
            + (wlo + whi) / 1000.0
        relabel = np.zeros(NW, np.int64)
        ra = np.argsort(-key[:NAW], kind="stable")
        relabel[ra] = np.arange(NAW)
        rb = np.argsort(-key[NAW:], kind="stable")
        relabel[NAW + rb] = NAW + np.arange(NW - NAW)
        win_of[c] = relabel[raw]
        for wi in range(NW):
            members = np.nonzero(win_of[c] == wi)[0]
            assert len(members) <= 32, (c, wi, len(members))
            col_of[c, members] = np.arange(len(members))

    pos_of = win_of * WCOL + col_of          # packed position per (core, node)

    # table row of each source node (packed coords)
    sc_pos = pos_of[sc_core, src % NSH]
    is_lo = sc_pos < ASIDE
    # sanity: is_lo must equal src_side_a
    assert np.array_equal(is_lo, src_side_a)
    tabrow = np.where(is_lo, sc_core * ASIDE + sc_pos,
                      sc_core * BSIDE + (sc_pos - ASIDE))

    # ---- per (core, window, region) counts -> capacities (max over cores)
    ew = win_of[score, nloc]                  # window of each edge (dst side)
    cnt_lo = np.zeros((NCORES, NW), np.int64)
    cnt_hi = np.zeros((NCORES, NW), np.int64)
    np.add.at(cnt_lo, (score[is_lo], ew[is_lo]), 1)
    np.add.at(cnt_hi, (score[~is_lo], ew[~is_lo]), 1)
    cap_lo = np.maximum(1, np.ceil(cnt_lo.max(axis=0) / CHUNK)).astype(np.int64)
    cap_hi = np.maximum(1, np.ceil(cnt_hi.max(axis=0) / CHUNK)).astype(np.int64)

    sb_windows = [list(range(s * SBW, min((s + 1) * SBW, NW))) for s in range(NSB)]
    CA_sb = [int(sum(cap_lo[w] for w in ws)) for ws in sb_windows]
    CB_sb = [int(sum(cap_hi[w] for w in ws)) for ws in sb_windows]
    C_sb = [a + b for a, b in zip(CA_sb, CB_sb)]
    choff = np.concatenate([[0], np.cumsum(C_sb)]).astype(np.int64)
    TOTCH = int(choff[-1])
    LA = sum(CA_sb) * CHUNK
    LB = sum(CB_sb) * CHUNK

    # chunk layout within sb: lo chunks (by window), then hi chunks
    blkA_off, blkB_off = [], []
    for s, ws in enumerate(sb_windows):
        offs, acc = {}, 0
        for w in ws:
            offs[w] = acc
            acc += int(cap_lo[w])
        blkA_off.append(offs)
        offs, acc = {}, 0
        for w in ws:
            offs[w] = acc
            acc += int(cap_hi[w])
        blkB_off.append(offs)

    # ---- per-core edge slot assignment
    ecol = col_of[score, nloc]
    percore = []
    for c in range(NCORES):
        m = score == c
        e_tab = tabrow[m]
        e_lo = is_lo[m]
        e_w = ew[m]
        e_col = ecol[m]
        e_nrm = norm[m]

        idxA = np.zeros(max(LA, 16), np.int16)
        idxB = np.zeros(max(LB, 16), np.int16)
        scoef = np.zeros((TOTCH, CHUNK, WCOL), np.float32)
        # fill counters per (window, region)
        fill = np.zeros((NW, 2), np.int64)
        order = np.argsort(e_w, kind="stable")
        for ei in order:
            w = e_w[ei]
            s = w // SBW
            lo = bool(e_lo[ei])
            k = fill[w, 0 if lo else 1]
            fill[w, 0 if lo else 1] += 1
            kk = k // CHUNK
            p = k % CHUNK
            if lo:
                assert kk < cap_lo[w], (c, w, kk)
                blk = blkA_off[s][w] + kk
                ch = int(choff[s]) + blk
                # A slot index within flat idxA: (sum of CA_sb before s +
                # blk)*CHUNK + p
                abase = sum(CA_sb[:s])
                idxA[(abase + blk) * CHUNK + p] = e_tab[ei]
            else:
                assert kk < cap_hi[w], (c, w, kk)
                blk = CA_sb[s] + blkB_off[s][w] + kk
                ch = int(choff[s]) + blk
                bbase = sum(CB_sb[:s])
                bblk = blkB_off[s][w] + kk
                idxB[(bbase + bblk) * CHUNK + p] = e_tab[ei]
            scoef[ch, p, e_col[ei]] = e_nrm[ei]

        def wrap_idx(flat):
            L = len(flat)
            assert L % 16 == 0
            w16 = flat.reshape(L // 16, 16).T.copy()
            return np.tile(w16, (8, 1))

        percore.append(dict(
            idxA=wrap_idx(idxA),
            idxB=wrap_idx(idxB),
            scoef=np.ascontiguousarray(
                scoef.transpose(1, 0, 2)).astype(mybir.dt.np(BF16)),
        ))

    meta = dict(cap_lo=cap_lo, cap_hi=cap_hi, sb_windows=sb_windows,
                CA_sb=CA_sb, CB_sb=CB_sb, C_sb=C_sb, choff=choff,
                TOTCH=TOTCH, LA=LA, LB=LB,
                blkA_off=blkA_off, blkB_off=blkB_off)
    aux = dict(pos_of=pos_of, selfco=selfco, dinv=dinv)
    return meta, percore, aux


# ---------------------------------------------------------------- build
def _build(meta):
    cap_lo, cap_hi = meta["cap_lo"], meta["cap_hi"]
    sb_windows = meta["sb_windows"]
    CA_sb, CB_sb, C_sb = meta["CA_sb"], meta["CB_sb"], meta["C_sb"]
    choff, TOTCH, LA, LB = meta["choff"], meta["TOTCH"], meta["LA"], meta["LB"]
    blkA_off, blkB_off = meta["blkA_off"], meta["blkB_off"]
    CSB_MAX = max(C_sb)

    nc = bacc.Bacc(num_devices=NCORES, num_swdge_queues=4,
                   dynamic_dma_scratch_size=SCRATCH)
    p_xT = nc.declare_dram_parameter("xT", [IN, PNSH], BF16, isOutput=False)
    p_idxA = nc.declare_dram_parameter("idxA", [128, max(LA, 16) // 16], I16,
                                       isOutput=False)
    p_idxB = nc.declare_dram_parameter("idxB", [128, max(LB, 16) // 16], I16,
                                       isOutput=False)
    p_sc = nc.declare_dram_parameter("scoef", [128, TOTCH, WCOL], BF16,
                                     isOutput=False)
    p_selfco = nc.declare_dram_parameter("selfcoT", [128, PNSH], BF16,
                                         isOutput=False)
    p_w0 = nc.declare_dram_parameter("w0", [IN, H], BF16, isOutput=False)
    p_b0 = nc.declare_dram_parameter("b0", [H, 1], F32, isOutput=False)
    p_aW1T = nc.declare_dram_parameter("aW1T", [H, H], F32, isOutput=False)
    p_gw1 = nc.declare_dram_parameter("gw1", [H, H], F32, isOutput=False)
    p_ba1 = nc.declare_dram_parameter("ba1", [H, 1], F32, isOutput=False)
    p_w2T = nc.declare_dram_parameter("w2T", [H, H2], F32, isOutput=False)
    p_b2 = nc.declare_dram_parameter("b2", [H2, 1], F32, isOutput=False)
    p_aW2T = nc.declare_dram_parameter("aW2T", [H2, H2], F32, isOutput=False)
    p_gw2 = nc.declare_dram_parameter("gw2", [H2, H2], F32, isOutput=False)
    p_ba2 = nc.declare_dram_parameter("ba2", [H2, 1], F32, isOutput=False)
    p_wfT = nc.declare_dram_parameter("wfT", [H2, OUT], F32, isOutput=False)
    p_bfc = nc.declare_dram_parameter("bfc", [128, OUT], F32, isOutput=False)
    p_ident = nc.declare_dram_parameter("ident", [128, 128], BF16, isOutput=False)
    p_out = nc.declare_dram_parameter("out", [PNSH, OUT], F32, isOutput=True)

    agA = [nc.dram_tensor(f"agA{i}", [ASIDE, H], BF16) for i in range(3)]
    agB = [nc.dram_tensor(f"agB{i}", [BSIDE, H], BF16) for i in range(3)]
    tabA = [nc.dram_tensor(f"tabA{i}", [NCORES * ASIDE, H], BF16,
                           addr_space="Shared") for i in range(3)]
    tabB = [nc.dram_tensor(f"tabB{i}", [NCORES * BSIDE, H], BF16,
                           addr_space="Shared") for i in range(3)]

    def wslice(s):
        lo = s * SBW * WCOL
        hi = min(PNSH, (s + 1) * SBW * WCOL)
        return lo, hi - lo

    IT1 = int(os.environ.get("K_IT1", "3"))
    IT2 = int(os.environ.get("K_IT2", "1"))

    with TileContext(nc) as tc:
        with (
            tc.tile_pool(name="const", bufs=1) as cp,
            tc.tile_pool(name="xin", bufs=3) as xp,
            tc.tile_pool(name="gat", bufs=3) as gp,
            tc.tile_pool(name="scp", bufs=2) as scp,
            tc.tile_pool(name="stg", bufs=2) as sp,
            tc.tile_pool(name="wrk", bufs=3) as wp,
            tc.tile_pool(name="pa", bufs=2, space="PSUM") as pa,
            tc.tile_pool(name="py", bufs=2, space="PSUM") as py,
            tc.tile_pool(name="pt", bufs=2, space="PSUM") as pt,
        ):
            hT = cp.tile([H, PNSH], F32, tag="hT")
            h2T = cp.tile([H2, PNSH], F32, tag="h2T")
            t_idxA = cp.tile([128, max(LA, 16) // 16], I16, tag="idxA")
            t_idxB = cp.tile([128, max(LB, 16) // 16], I16, tag="idxB")
            selfcoT = cp.tile([128, PNSH], BF16, tag="selfcoT")
            w0a = cp.tile([128, H], BF16, tag="w0a")
            w0b = cp.tile([128, H], BF16, tag="w0b")
            b0 = cp.tile([H, 1], F32, tag="b0")
            aW1T = cp.tile([H, H], F32, tag="aW1T")
            gw1 = cp.tile([H, H], F32, tag="gw1")
            ba1 = cp.tile([H, 1], F32, tag="ba1")
            w2T = cp.tile([H, H2], F32, tag="w2T")
            b2 = cp.tile([H2, 1], F32, tag="b2")
            aW2T = cp.tile([H2, H2], F32, tag="aW2T")
            gw2 = cp.tile([H2, H2], F32, tag="gw2")
            ba2 = cp.tile([H2, 1], F32, tag="ba2")
            wfT = cp.tile([H2, OUT], F32, tag="wfT")
            bfc = cp.tile([128, OUT], F32, tag="bfc")
            ident = cp.tile([128, 128], BF16, tag="ident")

            nc.sync.dma_start(out=t_idxA[:], in_=p_idxA[:, :])
            nc.sync.dma_start(out=t_idxB[:], in_=p_idxB[:, :])
            nc.sync.dma_start(out=selfcoT[:], in_=p_selfco[:, :])
            nc.sync.dma_start(out=w0a[:], in_=p_w0[0:128, :])
            nc.sync.dma_start(out=w0b[:], in_=p_w0[128:256, :])
            nc.sync.dma_start(out=b0[:], in_=p_b0[:, :])
            nc.sync.dma_start(out=aW1T[:], in_=p_aW1T[:, :])
            nc.sync.dma_start(out=gw1[:], in_=p_gw1[:, :])
            nc.sync.dma_start(out=ba1[:], in_=p_ba1[:, :])
            nc.sync.dma_start(out=w2T[:], in_=p_w2T[:, :])
            nc.sync.dma_start(out=b2[:], in_=p_b2[:, :])
            nc.sync.dma_start(out=aW2T[:], in_=p_aW2T[:, :])
            nc.sync.dma_start(out=gw2[:], in_=p_gw2[:, :])
            nc.sync.dma_start(out=ba2[:], in_=p_ba2[:, :])
            nc.sync.dma_start(out=wfT[:], in_=p_wfT[:, :])
            nc.sync.dma_start(out=bfc[:], in_=p_bfc[:, :])
            nc.sync.dma_start(out=ident[:], in_=p_ident[:, :])

            def stage_sb(src_t, srcdim, gwt, s, par):
                """table rows for sb s from state src_t: gw @ src ->
                bf16 -> transpose -> write to ag bufs (parity par)."""
                lo, n = wslice(s)
                ps = pa.tile([srcdim, 512], F32, tag="pa")
                nc.tensor.matmul(ps[:, :n], gwt[:], src_t[:, lo:lo + n],
                                 start=True, stop=True)
                stg = sp.tile([srcdim, 512], BF16, tag="stg")
                nc.scalar.activation(stg[:, :n], ps[:, :n], AF.Copy)
                nt = (n + 127) // 128
                for t in range(nt):
                    q = lo + t * 128
                    ptt = pt.tile([128, 128], BF16, tag="pt")
                    nc.tensor.transpose(ptt[:, :srcdim],
                                        stg[:, t * 128:t * 128 + 128],
                                        ident[:srcdim, :srcdim])
                    rows = sp.tile([128, 128], BF16, tag="rows")
                    nc.scalar.activation(rows[:, :srcdim], ptt[:, :srcdim],
                                         AF.Copy)
                    if q < ASIDE:
                        nc.sync.dma_start(out=agA[par][q:q + 128, 0:srcdim],
                                          in_=rows[:, :srcdim])
                    else:
                        qq = q - ASIDE
                        nc.sync.dma_start(out=agB[par][qq:qq + 128, 0:srcdim],
                                          in_=rows[:, :srcdim])

            def exchangeA(par):
                nc.gpsimd.collective_compute(
                    "AllGather", ALU.bypass,
                    replica_groups=[list(range(NCORES))],
                    ins=[agA[par][:, :]], outs=[tabA[par][:, :]])

            def exchangeB(par):
                nc.gpsimd.collective_compute(
                    "AllGather", ALU.bypass,
                    replica_groups=[list(range(NCORES))],
                    ins=[agB[par][:, :]], outs=[tabB[par][:, :]])

            def conv_iter(state_t, dim, aWt, gwt, bias_t, par, stage_next):
                """one antisymmetric conv step; stage_next = (gw_tile,
                parity) to stage the NEXT table from the updated state,
                or None."""
                offA = offB = 0
                for s in range(NSB):
                    lo, n = wslice(s)
                    ca, cb = CA_sb[s], CB_sb[s]
                    sct = scp.tile([128, CSB_MAX, WCOL], BF16, tag="sct")
                    nc.sync.dma_start(
                        out=sct[:, 0:C_sb[s], :],
                        in_=p_sc[:, int(choff[s]):int(choff[s + 1]), :])
                    g = gp.tile([128, CSB_MAX, 128], BF16, tag="g")
                    qn = 0
                    for b0_ in range(0, ca, GCAP):
                        b1 = min(ca, b0_ + GCAP)
                        o = offA + b0_ * CHUNK
                        nc.gpsimd.dma_gather(
                            out_ap=g[:, b0_:b1, :], in_ap=tabA[par][:, :],
                            idxs_ap=t_idxA[:, o // 16:(o + (b1 - b0_) * CHUNK) // 16],
                            num_idxs=(b1 - b0_) * CHUNK,
                            num_idxs_reg=(b1 - b0_) * CHUNK,
                            elem_size=H, queue_num=qn % 4)
                        qn += 1
                    for b0_ in range(0, cb, GCAP):
                        b1 = min(cb, b0_ + GCAP)
                        o = offB + b0_ * CHUNK
                        nc.gpsimd.dma_gather(
                            out_ap=g[:, ca + b0_:ca + b1, :], in_ap=tabB[par][:, :],
                            idxs_ap=t_idxB[:, o // 16:(o + (b1 - b0_) * CHUNK) // 16],
                            num_idxs=(b1 - b0_) * CHUNK,
                            num_idxs_reg=(b1 - b0_) * CHUNK,
                            elem_size=H, queue_num=qn % 4)
                        qn += 1
                    offA += ca * CHUNK
                    offB += cb * CHUNK

                    psy = py.tile([dim, 512], F32, tag="py")
                    first = True
                    for w in sb_windows[s]:
                        colb = (w % SBW) * WCOL
                        for k in range(int(cap_lo[w])):
                            blk = blkA_off[s][w] + k
                            nc.tensor.matmul(
                                psy[:, colb:colb + WCOL],
                                g[:, blk, 0:dim],
                                sct[:, blk, :],
                                start=first, stop=False,
                                skip_group_check=True)
                            first = False
                        for k in range(int(cap_hi[w])):
                            blk = ca + blkB_off[s][w] + k
                            nc.tensor.matmul(
                                psy[:, colb:colb + WCOL],
                                g[:, blk, 0:dim],
                                sct[:, blk, :],
                                start=first, stop=False,
                                skip_group_check=True)
                            first = False
                    # self-loop term: gw @ (selfco * h)
                    ssc = wp.tile([dim, 512], F32, tag="ssc")
                    nc.vector.tensor_tensor(ssc[:, :n], state_t[:, lo:lo + n],
                                            selfcoT[:dim, lo:lo + n], ALU.mult)
                    nc.tensor.matmul(psy[:, :n], gwt[:], ssc[:, :n],
                                     start=False, stop=False,
                                     skip_group_check=True)
                    nc.tensor.matmul(psy[:, :n], aWt[:], state_t[:, lo:lo + n],
                                     start=False, stop=True,
                                     skip_group_check=True)
                    upd = wp.tile([dim, 512], F32, tag="upd")
                    nc.scalar.activation(upd[:, :n], psy[:, :n], AF.Tanh,
                                         bias=bias_t[:, :])
                    nc.vector.scalar_tensor_tensor(
                        state_t[:, lo:lo + n], upd[:, :n], EPS,
                        state_t[:, lo:lo + n], ALU.mult, ALU.add)
                    if stage_next is not None:
                        gw_n, par_n = stage_next
                        stage_sb(state_t, dim, gw_n, s, par_n)
                        if s == 7:
                            exchangeA(par_n)
                        elif s == NSB - 1:
                            exchangeB(par_n)

            # ---- zero state (dead cols must be finite)
            nc.vector.memset(hT[:, :], 0)
            nc.vector.memset(h2T[:, :], 0)

            # ---- layer 0 + stage table 0 (parity 0)
            for s in range(NSB):
                lo, n = wslice(s)
                ps = pa.tile([H, 512], F32, tag="pa")
                for kc, w0t in enumerate((w0a, w0b)):
                    xt = xp.tile([128, 512], BF16, tag="xt")
                    nc.sync.dma_start(out=xt[:, :n],
                                      in_=p_xT[kc * 128:(kc + 1) * 128, lo:lo + n])
                    nc.tensor.matmul(ps[:, :n], w0t[:], xt[:, :n],
                                     start=(kc == 0), stop=(kc == 1))
                t0 = wp.tile([H, 512], F32, tag="t0")
                nc.scalar.activation(t0[:, :n], ps[:, :n], AF.Identity,
                                     bias=b0[:, :])
                nc.vector.scalar_tensor_tensor(hT[:, lo:lo + n], t0[:, :n], 0.01,
                                               t0[:, :n], ALU.mult, ALU.max)
                stage_sb(hT, H, gw1, s, 0)
                if s == 7:
                    exchangeA(0)
                elif s == NSB - 1:
                    exchangeB(0)

            # ---- conv1 x IT1
            for it in range(IT1):
                par = it % 3
                stage_next = (gw1, (it + 1) % 3) if it + 1 < IT1 else None
                conv_iter(hT, H, aW1T, gw1, ba1, par, stage_next)

            # ---- transition: g = lrelu(hT); h2T = lrelu(w_hid2 @ g + b2)
            # stage tab2 (parity IT1%2) fused per sb
            par2 = IT1 % 3
            for s in range(NSB):
                lo, n = wslice(s)
                gk = wp.tile([H, 512], F32, tag="tsum")
                nc.vector.scalar_tensor_tensor(gk[:, :n], hT[:, lo:lo + n], 0.01,
                                               hT[:, lo:lo + n], ALU.mult, ALU.max)
                ps = pa.tile([H2, 512], F32, tag="pa")
                nc.tensor.matmul(ps[:, :n], w2T[:], gk[:, :n], start=True,
                                 stop=True)
                t2 = wp.tile([H2, 512], F32, tag="upd")
                nc.scalar.activation(t2[:, :n], ps[:, :n], AF.Identity,
                                     bias=b2[:, :])
                nc.vector.scalar_tensor_tensor(h2T[:, lo:lo + n], t2[:, :n], 0.01,
                                               t2[:, :n], ALU.mult, ALU.max)
                if IT2 > 0:
                    stage_sb(h2T, H2, gw2, s, par2)
                    if s == 7:
                        exchangeA(par2)
                    elif s == NSB - 1:
                        exchangeB(par2)

            # ---- conv2
            if IT2 > 0:
                conv_iter(h2T, H2, aW2T, gw2, ba2, par2, None)

            # ---- final: logits + log_softmax, node-major
            NT = PNSH // 128
            for t in range(NT):
                pf = pa.tile([128, OUT], F32, tag="pd")
                nc.tensor.matmul(pf[:, :], h2T[:, t * 128:(t + 1) * 128],
                                 wfT[:], start=True, stop=True)
                lg = wp.tile([128, OUT], F32, tag="lg")
                nc.vector.tensor_tensor(lg[:, :], pf[:, :], bfc[:, :], ALU.add)
                nmx = wp.tile([128, 1], F32, tag="nmx")
                nc.vector.tensor_reduce(nmx[:, :], lg[:, :],
                                        mybir.AxisListType.X, ALU.max, negate=True)
                ex = wp.tile([128, OUT], F32, tag="ex")
                se = wp.tile([128, 1], F32, tag="se")
                nc.scalar.activation(ex[:, :], lg[:, :], AF.Exp,
                                     bias=nmx[:, :], accum_out=se[:, :])
                lse = wp.tile([128, 1], F32, tag="lse")
                nc.scalar.activation(lse[:, :], se[:, :], AF.Ln)
                shift = wp.tile([128, 1], F32, tag="shift")
                nc.vector.tensor_tensor(shift[:, :], nmx[:, :], lse[:, :],
                                        ALU.subtract)
                ot = wp.tile([128, OUT], F32, tag="ot")
                nc.vector.tensor_scalar(ot[:, :], lg[:, :], shift[:, :],
                                        None, ALU.add)
                nc.sync.dma_start(out=p_out[t * 128:(t + 1) * 128, :],
                                  in_=ot[:, :])

    nc.finalize()
    return nc


# ----------------------------------------------------------------- run
def kernel(x, edge_index, w_hid, b_hid, W_a1, gcn_w1, b_a1,
           w_hid2, b_hid2, W_a2, gcn_w2, b_a2, w_fc, b_fc, _trace=False):
    x = np.asarray(x, np.float32)
    meta, percore, aux = _prep_graph(edge_index)
    nc = _build(meta)
    pos_of, selfco = aux["pos_of"], aux["selfco"]

    f32 = np.float32
    bfnp = mybir.dt.np(BF16)
    aW1 = np.asarray(W_a1, f32)
    aW1T = np.ascontiguousarray(aW1.T - aW1 - GAMMA * np.eye(H, dtype=f32))
    aW2 = np.asarray(W_a2, f32)
    aW2T = np.ascontiguousarray(aW2.T - aW2 - GAMMA * np.eye(H2, dtype=f32))
    common = dict(
        w0=np.ascontiguousarray(np.asarray(w_hid, f32).T).astype(bfnp),
        b0=np.asarray(b_hid, f32).reshape(H, 1),
        aW1T=aW1T,
        gw1=np.ascontiguousarray(np.asarray(gcn_w1, f32)),
        ba1=np.asarray(b_a1, f32).reshape(H, 1),
        w2T=np.ascontiguousarray(np.asarray(w_hid2, f32).T),
        b2=np.asarray(b_hid2, f32).reshape(H2, 1),
        aW2T=aW2T,
        gw2=np.ascontiguousarray(np.asarray(gcn_w2, f32)),
        ba2=np.asarray(b_a2, f32).reshape(H2, 1),
        wfT=np.ascontiguousarray(np.asarray(w_fc, f32).T),
        bfc=np.tile(np.asarray(b_fc, f32).reshape(1, OUT), (128, 1)),
        ident=np.eye(128, dtype=bfnp),
    )
    in_maps = []
    for c in range(NCORES):
        xs = x[c * NSH:(c + 1) * NSH]                  # [NSH, IN]
        xP = np.zeros((PNSH, IN), f32)
        xP[pos_of[c]] = xs
        scP = np.zeros(PNSH, f32)
        scP[pos_of[c]] = selfco[c * NSH:(c + 1) * NSH]
        in_maps.append({
            "xT": np.ascontiguousarray(xP.T).astype(bfnp),
            "selfcoT": np.tile(scP[None, :], (128, 1)).astype(bfnp),
            **percore[c], **common,
        })

    res = run_bass_kernel_spmd(nc, in_maps, list(range(NCORES)), trace=_trace)
    out = np.zeros((N, OUT), np.float32)
    for c in range(NCORES):
        out[c * NSH:(c + 1) * NSH] = res.results[c]["out"][pos_of[c]]
    kernel.last_exec_time_ns = res.exec_time_ns
    kernel.last_results = res
    kernel.last_hdump = np.zeros((NCORES, H, NSH), np.float32)
    return out


# revision 7
# speedup vs baseline: 1.0831x; 1.0831x over previous
"""AntiSymmetricDGN on 8 TRN2 NeuronCores (Bass/Tile, SPMD) — v2.

Node-sharded graph parallel, per-core state TRANSPOSED in SBUF
[feat, pos] where pos is a PACKED node order (degree-aware window
packing, 196 windows x 32 cols = 6272 positions per core, 22 dead).

Per conv iteration:
  - table rows (node-major bf16 hw rows) staged per superblock into
    local ag bufs; two AllGathers -> shared tables A (positions <4096,
    8x4096 rows) and B (positions >=4096, 8x2176 rows).
  - SWDGE dma_gather by edge-slot (idx int16) -> g tiles [128,C,128].
  - segment-sum via PE chunk matmuls: lhsT = g[:, j, :] (128 edges x
    feat), rhs = scoef[:, j, :] (128 edges x 32 dst cols), accumulated
    per 512-col superblock in PSUM.
  - self-loop term dense: psy += gcn_w @ (selfco * h).
  - aW term, tanh, Euler update; next iteration's table staged in the
    same sb pass (gcn matmul + PE transpose + DMA).

Degree-aware packing targets (lo<=256, hi<=128|256) per window so
per-window chunk capacities are (2, 1|2) -> ~1-4% slot padding vs 23%
in v1. Self-loops excluded from the edge stream.
"""
import math
import os
import numpy as np

import concourse.bass as bass
from concourse import mybir, bacc
from concourse.bass_utils import run_bass_kernel_spmd
from concourse.tile import TileContext

# problem constants
N, E, IN, H, H2, OUT = 50000, 600000, 256, 128, 64, 40
EPS, GAMMA = 0.1, 0.1
NCORES = 8
NSH = N // NCORES          # 6250 real nodes per core
WCOL = 32
NW = 196                   # windows per core
PNSH = NW * WCOL           # 6272 packed positions
SBW = 16                   # windows per superblock
NSB = (NW + SBW - 1) // SBW    # 13 superblocks (last has 4 windows)
ASIDE = 128 * WCOL         # 4096 positions on side A
BSIDE = PNSH - ASIDE       # 2176 on side B
NAW = 128                  # A-side windows
CHUNK = 128

F32 = mybir.dt.float32
BF16 = mybir.dt.bfloat16
I16 = mybir.dt.int16
AF = mybir.ActivationFunctionType
ALU = mybir.AluOpType

GCAP = int(os.environ.get("K_GCAP", "8"))
SCRATCH = int(os.environ.get("K_SCRATCH", "49152"))


def _pack_side(deg_lo, deg_hi, nodes, nwin, cap_hi_list):
    """Pack `nodes` (indices) into nwin windows of <=32 nodes with
    per-window edge budgets (256 lo, cap_hi*128 hi). LPT greedy with
    normalized-min-room score + repair passes. Returns dict node->win."""
    nodes = np.asarray(nodes)
    dl_a = deg_lo[nodes].astype(np.float64)
    dh_a = deg_hi[nodes].astype(np.float64)
    order = np.argsort(-(dl_a + dh_a), kind="stable")
    tlo = np.full(nwin, 256.0)
    thi = np.array([128.0 * c for c in cap_hi_list], np.float64)
    rem_lo = tlo.copy()
    rem_hi = thi.copy()
    slots = np.full(nwin, 32, dtype=np.int64)
    assign = np.full(len(nodes), -1, np.int64)
    for i in order:
        dl, dh = dl_a[i], dh_a[i]
        rl = (rem_lo - dl) / tlo
        rh = (rem_hi - dh) / thi
        score = np.minimum(rl, rh)
        feas = (slots > 0) & (rem_lo >= dl) & (rem_hi >= dh)
        sc = np.where(feas, score, -np.inf)
        if not np.isfinite(sc).any():
            sc = np.where(slots > 0, score, -np.inf)
        w = int(np.argmax(sc))
        assign[i] = w
        rem_lo[w] -= dl
        rem_hi[w] -= dh
        slots[w] -= 1
    # repair: move offenders out of overloaded windows
    for _ in range(12):
        over = np.nonzero((rem_lo < 0) | (rem_hi < 0))[0]
        if len(over) == 0:
            break
        moved = False
        for w in over:
            members = np.nonzero(assign == w)[0]
            by_sz = members[np.argsort(-(dl_a[members] + dh_a[members]))]
            for i in by_sz:
                if rem_lo[w] >= 0 and rem_hi[w] >= 0:
                    break
                dl, dh = dl_a[i], dh_a[i]
                feas = (slots > 0) & (rem_lo >= dl) & (rem_hi >= dh)
                feas[w] = False
                if not feas.any():
                    continue
                cand = np.where(feas, np.minimum((rem_lo - dl) / tlo,
                                                 (rem_hi - dh) / thi), -np.inf)
                w2 = int(np.argmax(cand))
                assign[i] = w2
                rem_lo[w] += dl
                rem_hi[w] += dh
                slots[w] += 1
                rem_lo[w2] -= dl
                rem_hi[w2] -= dh
                slots[w2] -= 1
                moved = True
        if not moved:
            break
    return {int(nodes[i]): int(assign[i]) for i in range(len(nodes))}


def _prep_graph(edge_index):
    src = np.asarray(edge_index[0], dtype=np.int64)
    dst = np.asarray(edge_index[1], dtype=np.int64)
    loops = np.arange(N, dtype=np.int64)
    degL = np.bincount(np.concatenate([dst, loops]), minlength=N).astype(np.float32)
    dinv = (1.0 / np.sqrt(np.maximum(degL, 1e-12))).astype(np.float32)
    dinv[degL <= 0] = 0.0
    norm = (dinv[src] * dinv[dst]).astype(np.float32)
    selfco = (dinv * dinv).astype(np.float32)

    score = dst // NSH  # dst core of each edge
    nloc = dst % NSH
    sc_core = src // NSH

    # ---- round 1: pack by total degree to fix sides
    deg_tot = np.zeros((NCORES, NSH), np.int64)
    np.add.at(deg_tot, (score, nloc), 1)
    sideA = np.zeros((NCORES, NSH), bool)
    for c in range(NCORES):
        order = np.argsort(-deg_tot[c], kind="stable")
        rem = np.full(NW, 384.0)
        slots = np.full(NW, 32, np.int64)
        w_of = np.full(NSH, -1, np.int64)
        for n in order:
            d = deg_tot[c, n]
            s2 = np.where(slots > 0, rem - d, -np.inf)
            w = int(np.argmax(s2))
            w_of[n] = w
            rem[w] -= d
            slots[w] -= 1
        sideA[c] = w_of < NAW

    # edge lo-ness: src node's side on its core
    src_side_a = sideA[sc_core, src % NSH]

    # per-node (dst) lo/hi in-degree
    deg_lo = np.zeros((NCORES, NSH), np.int64)
    deg_hi = np.zeros((NCORES, NSH), np.int64)
    np.add.at(deg_lo, (score[src_side_a], nloc[src_side_a]), 1)
    np.add.at(deg_hi, (score[~src_side_a], nloc[~src_side_a]), 1)

    # hi-special windows: extra hi capacity. Estimate demand per side.
    spec_a = int(os.environ.get("K_SPECA", "8"))
    spec_b = int(os.environ.get("K_SPECB", "5"))
    caphi_a = [2 if w >= NAW - spec_a else 1 for w in range(NAW)]
    caphi_b = [2 if w >= (NW - NAW) - spec_b else 1 for w in range(NW - NAW)]

    # ---- round 2: pack each side with known lo/hi degrees
    win_of = np.full((NCORES, NSH), -1, np.int64)
    col_of = np.full((NCORES, NSH), -1, np.int64)
    for c in range(NCORES):
        na = _pack_side(deg_lo[c], deg_hi[c], np.nonzero(sideA[c])[0], NAW, caphi_a)
        nb = _pack_side(deg_lo[c], deg_hi[c], np.nonzero(~sideA[c])[0],
                        NW - NAW, caphi_b)
        raw = np.full(NSH, -1, np.int64)
        for n, w in na.items():
            raw[n] = w
        for n, w in nb.items():
            raw[n] = NAW + w
        # relabel windows within each side so per-core capacity profiles
        # align across cores (sort by needed chunks desc)
        wlo = np.zeros(NW)
        whi = np.zeros(NW)
        np.add.at(wlo, raw, deg_lo[c])
        np.add.at(whi, raw, deg_hi[c])
        key = (np.ceil(wlo / CHUNK) * 100 + np.ceil(whi / CHUNK)) * 1000 \
            + (wlo + whi) / 1000.0
        relabel = np.zeros(NW, np.int64)
        ra = np.argsort(-key[:NAW], kind="stable")
        relabel[ra] = np.arange(NAW)
        rb = np.argsort(-key[NAW:], kind="stable")
        relabel[NAW + rb] = NAW + np.arange(NW - NAW)
        win_of[c] = relabel[raw]
        for wi in range(NW):
            members = np.nonzero(win_of[c] == wi)[0]
            assert len(members) <= 32, (c, wi, len(members))
            col_of[c, members] = np.arange(len(members))

    pos_of = win_of * WCOL + col_of          # packed position per (core, node)

    # table row of each source node (packed coords)
    sc_pos = pos_of[sc_core, src % NSH]
    is_lo = sc_pos < ASIDE
    # sanity: is_lo must equal src_side_a
    assert np.array_equal(is_lo, src_side_a)
    tabrow = np.where(is_lo, sc_core * ASIDE + sc_pos,
                      sc_core * BSIDE + (sc_pos - ASIDE))

    # ---- per (core, window, region) counts -> capacities (max over cores)
    ew = win_of[score, nloc]                  # window of each edge (dst side)
    cnt_lo = np.zeros((NCORES, NW), np.int64)
    cnt_hi = np.zeros((NCORES, NW), np.int64)
    np.add.at(cnt_lo, (score[is_lo], ew[is_lo]), 1)
    np.add.at(cnt_hi, (score[~is_lo], ew[~is_lo]), 1)
    cap_lo = np.maximum(1, np.ceil(cnt_lo.max(axis=0) / CHUNK)).astype(np.int64)
    cap_hi = np.maximum(1, np.ceil(cnt_hi.max(axis=0) / CHUNK)).astype(np.int64)

    sb_windows = [list(range(s * SBW, min((s + 1) * SBW, NW))) for s in range(NSB)]
    CA_sb = [int(sum(cap_lo[w] for w in ws)) for ws in sb_windows]
    CB_sb = [int(sum(cap_hi[w] for w in ws)) for ws in sb_windows]
    C_sb = [a + b for a, b in zip(CA_sb, CB_sb)]
    choff = np.concatenate([[0], np.cumsum(C_sb)]).astype(np.int64)
    TOTCH = int(choff[-1])
    LA = sum(CA_sb) * CHUNK
    LB = sum(CB_sb) * CHUNK

    # chunk layout within sb: lo chunks (by window), then hi chunks
    blkA_off, blkB_off = [], []
    for s, ws in enumerate(sb_windows):
        offs, acc = {}, 0
        for w in ws:
            offs[w] = acc
            acc += int(cap_lo[w])
        blkA_off.append(offs)
        offs, acc = {}, 0
        for w in ws:
            offs[w] = acc
            acc += int(cap_hi[w])
        blkB_off.append(offs)

    # ---- per-core edge slot assignment
    ecol = col_of[score, nloc]
    percore = []
    for c in range(NCORES):
        m = score == c
        e_tab = tabrow[m]
        e_lo = is_lo[m]
        e_w = ew[m]
        e_col = ecol[m]
        e_nrm = norm[m]

        idxA = np.zeros(max(LA, 16), np.int16)
        idxB = np.zeros(max(LB, 16), np.int16)
        scoef = np.zeros((TOTCH, CHUNK, WCOL), np.float32)
        # fill counters per (window, region)
        fill = np.zeros((NW, 2), np.int64)
        order = np.argsort(e_w, kind="stable")
        for ei in order:
            w = e_w[ei]
            s = w // SBW
            lo = bool(e_lo[ei])
            k = fill[w, 0 if lo else 1]
            fill[w, 0 if lo else 1] += 1
            kk = k // CHUNK
            p = k % CHUNK
            if lo:
                assert kk < cap_lo[w], (c, w, kk)
                blk = blkA_off[s][w] + kk
                ch = int(choff[s]) + blk
                # A slot index within flat idxA: (sum of CA_sb before s +
                # blk)*CHUNK + p
                abase = sum(CA_sb[:s])
                idxA[(abase + blk) * CHUNK + p] = e_tab[ei]
            else:
                assert kk < cap_hi[w], (c, w, kk)
                blk = CA_sb[s] + blkB_off[s][w] + kk
                ch = int(choff[s]) + blk
                bbase = sum(CB_sb[:s])
                bblk = blkB_off[s][w] + kk
                idxB[(bbase + bblk) * CHUNK + p] = e_tab[ei]
            scoef[ch, p, e_col[ei]] = e_nrm[ei]

        def wrap_idx(flat):
            L = len(flat)
            assert L % 16 == 0
            w16 = flat.reshape(L // 16, 16).T.copy()
            return np.tile(w16, (8, 1))

        percore.append(dict(
            idxA=wrap_idx(idxA),
            idxB=wrap_idx(idxB),
            scoef=np.ascontiguousarray(
                scoef.transpose(1, 0, 2)).astype(mybir.dt.np(BF16)),
        ))

    meta = dict(cap_lo=cap_lo, cap_hi=cap_hi, sb_windows=sb_windows,
                CA_sb=CA_sb, CB_sb=CB_sb, C_sb=C_sb, choff=choff,
                TOTCH=TOTCH, LA=LA, LB=LB,
                blkA_off=blkA_off, blkB_off=blkB_off)
    aux = dict(pos_of=pos_of, selfco=selfco, dinv=dinv)
    return meta, percore, aux


# ---------------------------------------------------------------- build
def _build(meta):
    cap_lo, cap_hi = meta["cap_lo"], meta["cap_hi"]
    sb_windows = meta["sb_windows"]
    CA_sb, CB_sb, C_sb = meta["CA_sb"], meta["CB_sb"], meta["C_sb"]
    choff, TOTCH, LA, LB = meta["choff"], meta["TOTCH"], meta["LA"], meta["LB"]
    blkA_off, blkB_off = meta["blkA_off"], meta["blkB_off"]
    CSB_MAX = max(C_sb)

    nc = bacc.Bacc(num_devices=NCORES, num_swdge_queues=4,
                   dynamic_dma_scratch_size=SCRATCH)
    p_xT = nc.declare_dram_parameter("xT", [IN, PNSH], BF16, isOutput=False)
    p_idxA = nc.declare_dram_parameter("idxA", [128, max(LA, 16) // 16], I16,
                                       isOutput=False)
    p_idxB = nc.declare_dram_parameter("idxB", [128, max(LB, 16) // 16], I16,
                                       isOutput=False)
    p_sc = nc.declare_dram_parameter("scoef", [128, TOTCH, WCOL], BF16,
                                     isOutput=False)
    p_selfco = nc.declare_dram_parameter("selfcoT", [128, PNSH], BF16,
                                         isOutput=False)
    p_w0 = nc.declare_dram_parameter("w0", [IN, H], BF16, isOutput=False)
    p_b0 = nc.declare_dram_parameter("b0", [H, 1], F32, isOutput=False)
    p_aW1T = nc.declare_dram_parameter("aW1T", [H, H], F32, isOutput=False)
    p_gw1 = nc.declare_dram_parameter("gw1", [H, H], F32, isOutput=False)
    p_ba1 = nc.declare_dram_parameter("ba1", [H, 1], F32, isOutput=False)
    p_w2T = nc.declare_dram_parameter("w2T", [H, H2], F32, isOutput=False)
    p_b2 = nc.declare_dram_parameter("b2", [H2, 1], F32, isOutput=False)
    p_aW2T = nc.declare_dram_parameter("aW2T", [H2, H2], F32, isOutput=False)
    p_gw2 = nc.declare_dram_parameter("gw2", [H2, H2], F32, isOutput=False)
    p_ba2 = nc.declare_dram_parameter("ba2", [H2, 1], F32, isOutput=False)
    p_wfT = nc.declare_dram_parameter("wfT", [H2, OUT], F32, isOutput=False)
    p_bfc = nc.declare_dram_parameter("bfc", [128, OUT], F32, isOutput=False)
    p_ident = nc.declare_dram_parameter("ident", [128, 128], BF16, isOutput=False)
    p_out = nc.declare_dram_parameter("out", [PNSH, OUT], F32, isOutput=True)

    agA = [nc.dram_tensor(f"agA{i}", [ASIDE, H], BF16) for i in range(3)]
    agB = [nc.dram_tensor(f"agB{i}", [BSIDE, H], BF16) for i in range(3)]
    tabA = [nc.dram_tensor(f"tabA{i}", [NCORES * ASIDE, H], BF16,
                           addr_space="Shared") for i in range(3)]
    tabB = [nc.dram_tensor(f"tabB{i}", [NCORES * BSIDE, H], BF16,
                           addr_space="Shared") for i in range(3)]

    def wslice(s):
        lo = s * SBW * WCOL
        hi = min(PNSH, (s + 1) * SBW * WCOL)
        return lo, hi - lo

    IT1 = int(os.environ.get("K_IT1", "3"))
    IT2 = int(os.environ.get("K_IT2", "1"))

    with TileContext(nc) as tc:
        with (
            tc.tile_pool(name="const", bufs=1) as cp,
            tc.tile_pool(name="xin", bufs=3) as xp,
            tc.tile_pool(name="gat", bufs=3) as gp,
            tc.tile_pool(name="scp", bufs=2) as scp,
            tc.tile_pool(name="stg", bufs=2) as sp,
            tc.tile_pool(name="wrk", bufs=3) as wp,
            tc.tile_pool(name="pa", bufs=2, space="PSUM") as pa,
            tc.tile_pool(name="py", bufs=2, space="PSUM") as py,
            tc.tile_pool(name="pt", bufs=2, space="PSUM") as pt,
        ):
            hT = cp.tile([H, PNSH], F32, tag="hT")
            h2T = cp.tile([H2, PNSH], F32, tag="h2T")
            t_idxA = cp.tile([128, max(LA, 16) // 16], I16, tag="idxA")
            t_idxB = cp.tile([128, max(LB, 16) // 16], I16, tag="idxB")
            selfcoT = cp.tile([128, PNSH], BF16, tag="selfcoT")
            w0a = cp.tile([128, H], BF16, tag="w0a")
            w0b = cp.tile([128, H], BF16, tag="w0b")
            b0 = cp.tile([H, 1], F32, tag="b0")
            aW1T = cp.tile([H, H], F32, tag="aW1T")
            gw1 = cp.tile([H, H], F32, tag="gw1")
            ba1 = cp.tile([H, 1], F32, tag="ba1")
            w2T = cp.tile([H, H2], F32, tag="w2T")
            b2 = cp.tile([H2, 1], F32, tag="b2")
            aW2T = cp.tile([H2, H2], F32, tag="aW2T")
            gw2 = cp.tile([H2, H2], F32, tag="gw2")
            ba2 = cp.tile([H2, 1], F32, tag="ba2")
            wfT = cp.tile([H2, OUT], F32, tag="wfT")
            bfc = cp.tile([128, OUT], F32, tag="bfc")
            ident = cp.tile([128, 128], BF16, tag="ident")

            nc.sync.dma_start(out=t_idxA[:], in_=p_idxA[:, :])
            nc.sync.dma_start(out=t_idxB[:], in_=p_idxB[:, :])
            nc.sync.dma_start(out=selfcoT[:], in_=p_selfco[:, :])
            nc.sync.dma_start(out=w0a[:], in_=p_w0[0:128, :])
            nc.sync.dma_start(out=w0b[:], in_=p_w0[128:256, :])
            nc.sync.dma_start(out=b0[:], in_=p_b0[:, :])
            nc.sync.dma_start(out=aW1T[:], in_=p_aW1T[:, :])
            nc.sync.dma_start(out=gw1[:], in_=p_gw1[:, :])
            nc.sync.dma_start(out=ba1[:], in_=p_ba1[:, :])
            nc.sync.dma_start(out=w2T[:], in_=p_w2T[:, :])
            nc.sync.dma_start(out=b2[:], in_=p_b2[:, :])
            nc.sync.dma_start(out=aW2T[:], in_=p_aW2T[:, :])
            nc.sync.dma_start(out=gw2[:], in_=p_gw2[:, :])
            nc.sync.dma_start(out=ba2[:], in_=p_ba2[:, :])
            nc.sync.dma_start(out=wfT[:], in_=p_wfT[:, :])
            nc.sync.dma_start(out=bfc[:], in_=p_bfc[:, :])
            nc.sync.dma_start(out=ident[:], in_=p_ident[:, :])

            def stage_sb(src_t, srcdim, gwt, s, par):
                """table rows for sb s from state src_t: gw @ src ->
                bf16 -> transpose -> write to ag bufs (parity par)."""
                lo, n = wslice(s)
                ps = pa.tile([srcdim, 512], F32, tag="pa")
                nc.tensor.matmul(ps[:, :n], gwt[:], src_t[:, lo:lo + n],
                                 start=True, stop=True)
                stg = sp.tile([srcdim, 512], BF16, tag="stg")
                nc.scalar.activation(stg[:, :n], ps[:, :n], AF.Copy)
                nt = (n + 127) // 128
                for t in range(nt):
                    q = lo + t * 128
                    ptt = pt.tile([128, 128], BF16, tag="pt")
                    nc.tensor.transpose(ptt[:, :srcdim],
                                        stg[:, t * 128:t * 128 + 128],
                                        ident[:srcdim, :srcdim])
                    rows = sp.tile([128, 128], BF16, tag="rows")
                    nc.scalar.activation(rows[:, :srcdim], ptt[:, :srcdim],
                                         AF.Copy)
                    if q < ASIDE:
                        nc.sync.dma_start(out=agA[par][q:q + 128, 0:srcdim],
                                          in_=rows[:, :srcdim])
                    else:
                        qq = q - ASIDE
                        nc.sync.dma_start(out=agB[par][qq:qq + 128, 0:srcdim],
                                          in_=rows[:, :srcdim])

            def exchangeA(par):
                nc.gpsimd.collective_compute(
                    "AllGather", ALU.bypass,
                    replica_groups=[list(range(NCORES))],
                    ins=[agA[par][:, :]], outs=[tabA[par][:, :]])

            def exchangeB(par):
                nc.gpsimd.collective_compute(
                    "AllGather", ALU.bypass,
                    replica_groups=[list(range(NCORES))],
                    ins=[agB[par][:, :]], outs=[tabB[par][:, :]])

            def conv_iter(state_t, dim, aWt, gwt, bias_t, par, stage_next,
                          exchange_b_par=None):
                """one antisymmetric conv step; stage_next = (gw_tile,
                parity) to stage the NEXT table from the updated state,
                or None."""
                offA = offB = 0
                for s in range(NSB):
                    lo, n = wslice(s)
                    ca, cb = CA_sb[s], CB_sb[s]
                    sct = scp.tile([128, CSB_MAX, WCOL], BF16, tag="sct")
                    nc.sync.dma_start(
                        out=sct[:, 0:C_sb[s], :],
                        in_=p_sc[:, int(choff[s]):int(choff[s + 1]), :])
                    g = gp.tile([128, CSB_MAX, 128], BF16, tag="g")
                    qn = 0
                    for b0_ in range(0, ca, GCAP):
                        b1 = min(ca, b0_ + GCAP)
                        o = offA + b0_ * CHUNK
                        nc.gpsimd.dma_gather(
                            out_ap=g[:, b0_:b1, :], in_ap=tabA[par][:, :],
                            idxs_ap=t_idxA[:, o // 16:(o + (b1 - b0_) * CHUNK) // 16],
                            num_idxs=(b1 - b0_) * CHUNK,
                            num_idxs_reg=(b1 - b0_) * CHUNK,
                            elem_size=H, queue_num=qn % 4)
                        qn += 1
                    if s == 0 and exchange_b_par is not None:
                        exchangeB(exchange_b_par)
                    for b0_ in range(0, cb, GCAP):
                        b1 = min(cb, b0_ + GCAP)
                        o = offB + b0_ * CHUNK
                        nc.gpsimd.dma_gather(
                            out_ap=g[:, ca + b0_:ca + b1, :], in_ap=tabB[par][:, :],
                            idxs_ap=t_idxB[:, o // 16:(o + (b1 - b0_) * CHUNK) // 16],
                            num_idxs=(b1 - b0_) * CHUNK,
                            num_idxs_reg=(b1 - b0_) * CHUNK,
                            elem_size=H, queue_num=qn % 4)
                        qn += 1
                    offA += ca * CHUNK
                    offB += cb * CHUNK

                    psy = py.tile([dim, 512], F32, tag="py")
                    first = True
                    for w in sb_windows[s]:
                        colb = (w % SBW) * WCOL
                        for k in range(int(cap_lo[w])):
                            blk = blkA_off[s][w] + k
                            nc.tensor.matmul(
                                psy[:, colb:colb + WCOL],
                                g[:, blk, 0:dim],
                                sct[:, blk, :],
                                start=first, stop=False,
                                skip_group_check=True)
                            first = False
                        for k in range(int(cap_hi[w])):
                            blk = ca + blkB_off[s][w] + k
                            nc.tensor.matmul(
                                psy[:, colb:colb + WCOL],
                                g[:, blk, 0:dim],
                                sct[:, blk, :],
                                start=first, stop=False,
                                skip_group_check=True)
                            first = False
                    # self-loop term: gw @ (selfco * h)
                    ssc = wp.tile([dim, 512], F32, tag="ssc")
                    nc.vector.tensor_tensor(ssc[:, :n], state_t[:, lo:lo + n],
                                            selfcoT[:dim, lo:lo + n], ALU.mult)
                    nc.tensor.matmul(psy[:, :n], gwt[:], ssc[:, :n],
                                     start=False, stop=False,
                                     skip_group_check=True)
                    nc.tensor.matmul(psy[:, :n], aWt[:], state_t[:, lo:lo + n],
                                     start=False, stop=True,
                                     skip_group_check=True)
                    upd = wp.tile([dim, 512], F32, tag="upd")
                    nc.scalar.activation(upd[:, :n], psy[:, :n], AF.Tanh,
                                         bias=bias_t[:, :])
                    nc.vector.scalar_tensor_tensor(
                        state_t[:, lo:lo + n], upd[:, :n], EPS,
                        state_t[:, lo:lo + n], ALU.mult, ALU.add)
                    if stage_next is not None:
                        gw_n, par_n = stage_next
                        stage_sb(state_t, dim, gw_n, s, par_n)

            # ---- zero state (dead cols must be finite)
            nc.vector.memset(hT[:, :], 0)
            nc.vector.memset(h2T[:, :], 0)

            # ---- layer 0 + stage table 0 (parity 0)
            for s in range(NSB):
                lo, n = wslice(s)
                ps = pa.tile([H, 512], F32, tag="pa")
                for kc, w0t in enumerate((w0a, w0b)):
                    xt = xp.tile([128, 512], BF16, tag="xt")
                    nc.sync.dma_start(out=xt[:, :n],
                                      in_=p_xT[kc * 128:(kc + 1) * 128, lo:lo + n])
                    nc.tensor.matmul(ps[:, :n], w0t[:], xt[:, :n],
                                     start=(kc == 0), stop=(kc == 1))
                t0 = wp.tile([H, 512], F32, tag="t0")
                nc.scalar.activation(t0[:, :n], ps[:, :n], AF.Identity,
                                     bias=b0[:, :])
                nc.vector.scalar_tensor_tensor(hT[:, lo:lo + n], t0[:, :n], 0.01,
                                               t0[:, :n], ALU.mult, ALU.max)
                stage_sb(hT, H, gw1, s, 0)

            # ---- conv1 x IT1
            for it in range(IT1):
                par = it % 3
                exchangeA(par)
                stage_next = (gw1, (it + 1) % 3) if it + 1 < IT1 else None
                conv_iter(hT, H, aW1T, gw1, ba1, par, stage_next,
                          exchange_b_par=par)

            # ---- transition: g = lrelu(hT); h2T = lrelu(w_hid2 @ g + b2)
            # stage tab2 (parity IT1%2) fused per sb
            par2 = IT1 % 3
            for s in range(NSB):
                lo, n = wslice(s)
                gk = wp.tile([H, 512], F32, tag="tsum")
                nc.vector.scalar_tensor_tensor(gk[:, :n], hT[:, lo:lo + n], 0.01,
                                               hT[:, lo:lo + n], ALU.mult, ALU.max)
                ps = pa.tile([H2, 512], F32, tag="pa")
                nc.tensor.matmul(ps[:, :n], w2T[:], gk[:, :n], start=True,
                                 stop=True)
                t2 = wp.tile([H2, 512], F32, tag="upd")
                nc.scalar.activation(t2[:, :n], ps[:, :n], AF.Identity,
                                     bias=b2[:, :])
                nc.vector.scalar_tensor_tensor(h2T[:, lo:lo + n], t2[:, :n], 0.01,
                                               t2[:, :n], ALU.mult, ALU.max)
                if IT2 > 0:
                    stage_sb(h2T, H2, gw2, s, par2)

            # ---- conv2
            if IT2 > 0:
                exchangeA(par2)
                conv_iter(h2T, H2, aW2T, gw2, ba2, par2, None,
                          exchange_b_par=par2)

            # ---- final: logits + log_softmax, node-major
            NT = PNSH // 128
            for t in range(NT):
                pf = pa.tile([128, OUT], F32, tag="pd")
                nc.tensor.matmul(pf[:, :], h2T[:, t * 128:(t + 1) * 128],
                                 wfT[:], start=True, stop=True)
                lg = wp.tile([128, OUT], F32, tag="lg")
                nc.vector.tensor_tensor(lg[:, :], pf[:, :], bfc[:, :], ALU.add)
                nmx = wp.tile([128, 1], F32, tag="nmx")
                nc.vector.tensor_reduce(nmx[:, :], lg[:, :],
                                        mybir.AxisListType.X, ALU.max, negate=True)
                ex = wp.tile([128, OUT], F32, tag="ex")
                se = wp.tile([128, 1], F32, tag="se")
                nc.scalar.activation(ex[:, :], lg[:, :], AF.Exp,
                                     bias=nmx[:, :], accum_out=se[:, :])
                lse = wp.tile([128, 1], F32, tag="lse")
                nc.scalar.activation(lse[:, :], se[:, :], AF.Ln)
                shift = wp.tile([128, 1], F32, tag="shift")
                nc.vector.tensor_tensor(shift[:, :], nmx[:, :], lse[:, :],
                                        ALU.subtract)
                ot = wp.tile([128, OUT], F32, tag="ot")
                nc.vector.tensor_scalar(ot[:, :], lg[:, :], shift[:, :],
                                        None, ALU.add)
                nc.sync.dma_start(out=p_out[t * 128:(t + 1) * 128, :],
                                  in_=ot[:, :])

    nc.finalize()
    return nc


# ----------------------------------------------------------------- run
def kernel(x, edge_index, w_hid, b_hid, W_a1, gcn_w1, b_a1,
           w_hid2, b_hid2, W_a2, gcn_w2, b_a2, w_fc, b_fc, _trace=False):
    x = np.asarray(x, np.float32)
    meta, percore, aux = _prep_graph(edge_index)
    nc = _build(meta)
    pos_of, selfco = aux["pos_of"], aux["selfco"]

    f32 = np.float32
    bfnp = mybir.dt.np(BF16)
    aW1 = np.asarray(W_a1, f32)
    aW1T = np.ascontiguousarray(aW1.T - aW1 - GAMMA * np.eye(H, dtype=f32))
    aW2 = np.asarray(W_a2, f32)
    aW2T = np.ascontiguousarray(aW2.T - aW2 - GAMMA * np.eye(H2, dtype=f32))
    common = dict(
        w0=np.ascontiguousarray(np.asarray(w_hid, f32).T).astype(bfnp),
        b0=np.asarray(b_hid, f32).reshape(H, 1),
        aW1T=aW1T,
        gw1=np.ascontiguousarray(np.asarray(gcn_w1, f32)),
        ba1=np.asarray(b_a1, f32).reshape(H, 1),
        w2T=np.ascontiguousarray(np.asarray(w_hid2, f32).T),
        b2=np.asarray(b_hid2, f32).reshape(H2, 1),
        aW2T=aW2T,
        gw2=np.ascontiguousarray(np.asarray(gcn_w2, f32)),
        ba2=np.asarray(b_a2, f32).reshape(H2, 1),
        wfT=np.ascontiguousarray(np.asarray(w_fc, f32).T),
        bfc=np.tile(np.asarray(b_fc, f32).reshape(1, OUT), (128, 1)),
        ident=np.eye(128, dtype=bfnp),
    )
    in_maps = []
    for c in range(NCORES):
        xs = x[c * NSH:(c + 1) * NSH]                  # [NSH, IN]
        xP = np.zeros((PNSH, IN), f32)
        xP[pos_of[c]] = xs
        scP = np.zeros(PNSH, f32)
        scP[pos_of[c]] = selfco[c * NSH:(c + 1) * NSH]
        in_maps.append({
            "xT": np.ascontiguousarray(xP.T).astype(bfnp),
            "selfcoT": np.tile(scP[None, :], (128, 1)).astype(bfnp),
            **percore[c], **common,
        })

    res = run_bass_kernel_spmd(nc, in_maps, list(range(NCORES)), trace=_trace)
    out = np.zeros((N, OUT), np.float32)
    for c in range(NCORES):
        out[c * NSH:(c + 1) * NSH] = res.results[c]["out"][pos_of[c]]
    kernel.last_exec_time_ns = res.exec_time_ns
    kernel.last_results = res
    kernel.last_hdump = np.zeros((NCORES, H, NSH), np.float32)
    return out
